# revision 27
# baseline (speedup 1.0000x reference)
"""Trainium2 Bass kernel for CausalSelectiveSelfAttentionForInference.

Sharding: 8 cores = 2 batches x 4 head-groups (3 heads each). Each core:
  - projects q,k (transposed [D, T] layout, head-pair packed) and v (bf16)
  - computes the head-0 selection path: att0^T -> S^T -> FF^T (exclusive
    cumsum over queries via tensor_tensor_scan) -> expNegM = exp(-FF)
  - per head: att^T (PE, h0/h1 quadrant-packed) -> exp (ACT) -> * expNegM
    (DVE) staged to SBUF, then AV as one clean PSUM accumulation chain with
    an appended ones-row for the softmax sums (PE)
  - normalizes and applies its w_proj row-slice -> partial out^T [768, 2048]
Host sums the 4 partials per batch and transposes.

The reference's top-k keep mask is numerically subsumed by softmax(att - FF):
pruned keys sit at FF >= ~50 above the kept mass, i.e. softmax weight ~e^-50.
Masking therefore reduces to the causal mask, which this kernel applies
exactly by only computing/consuming the causal column range of each key tile
(plus inclusive/strict triangle masks on the diagonal 128-blocks).

wqk column layout (built host-side, 512 cols = 4 m-tiles of 128):
  mt0 [q_h0 | q_h1] -> qTp   mt1 [k_h0 | k_h1] -> kTp
  mt2 [q_h2 | q_0 ] -> qX    mt3 [k_h2 | k_0 ] -> kX
so h0/h1 att matmuls pack into PE row-quadrants (0,0)/(64,0), and the
FF path (q0/k0) reads partition-base-64 slices of qX/kX. FF att0 matmuls
(rows 64-127) are emitted adjacent to h2 att matmuls (rows 0-63) so the two
64-contraction streams overlap in the PE's row-band quadrants.

v bias is folded into the output-projection bias row: y_norm picks up +b_v,
and out = y W + (b_v W + b_proj) is realized by a ones-row appended to ytn
(contraction row 192) whose matching w_proj row carries the combined bias.
"""

import math
import os
import sys

import numpy as np

for _p in ("/opt/trn_rl_repo",):
    if _p not in sys.path:
        sys.path.insert(0, _p)

import ml_dtypes

import concourse.bass as bass
import concourse.mybir as mybir
from concourse import bacc
from concourse import tile
from concourse.bass_utils import run_bass_kernel_spmd

BF16 = mybir.dt.bfloat16
F32 = mybir.dt.float32
AF = mybir.ActivationFunctionType
OP = mybir.AluOpType

B, T, C = 2, 2048, 768
H, D = 12, 64
HPG = 3            # heads per group (per core)
G = 4              # head groups per batch
N_CORES = 8
CT = 6             # contraction tiles for C=768
KT = T // 128      # 16 key tiles
NQ = T // 512      # 4 query chunks

_CACHED = {}


def build_program():
    nc = bacc.Bacc(None, target_bir_lowering=False)

    xt_d = nc.declare_dram_parameter("xt", [128, CT, T], BF16, isOutput=False)
    wqk_d = nc.declare_dram_parameter("wqk", [128, CT, 512], BF16, isOutput=False)
    bqk_d = nc.declare_dram_parameter("bqk", [128, 4], F32, isOutput=False)
    wv_d = nc.declare_dram_parameter("wv", [128, CT, HPG * D], BF16, isOutput=False)
    wp_d = nc.declare_dram_parameter("wp", [128, 2, C], BF16, isOutput=False)
    tris_d = nc.declare_dram_parameter("tris", [128, 512], BF16, isOutput=False)
    trie_d = nc.declare_dram_parameter("trie", [128, 128], BF16, isOutput=False)
    out_d = nc.declare_dram_parameter("out", [C, T], F32, isOutput=True)

    with tile.TileContext(nc) as tc:
        with (
            tc.tile_pool(name="const", bufs=1) as cpool,
            tc.tile_pool(name="big", bufs=1) as bigpool,
            tc.tile_pool(name="psA", bufs=2, space=bass.MemorySpace.PSUM) as psA,
            tc.tile_pool(name="psY", bufs=2, space=bass.MemorySpace.PSUM) as psY,
            tc.tile_pool(name="psV", bufs=2, space=bass.MemorySpace.PSUM) as psV,
        ):
            # ---- load inputs ----
            wqk = cpool.tile([128, CT, 512], BF16, tag="wqk")
            bqk = cpool.tile([128, 4], F32, tag="bqk")
            wv = cpool.tile([128, CT, HPG * D], BF16, tag="wv")
            wp = cpool.tile([128, 2, C], BF16, tag="wp")
            tris = cpool.tile([128, 512], BF16, tag="tris")
            trie = cpool.tile([128, 128], BF16, tag="trie")

            # pair-packed projections: [128, T] each (see module docstring)
            qTp = bigpool.tile([128, T], BF16, tag="qTp")
            kTp = bigpool.tile([128, T], BF16, tag="kTp")
            qX = bigpool.tile([128, T], BF16, tag="qX")
            kX = bigpool.tile([128, T], BF16, tag="kX")
            vaug = bigpool.tile([128, KT, HPG, 65], BF16, tag="vaug")
            nc.gpsimd.memset(vaug[:], 1.0)
            expnegm = bigpool.tile([128, KT, T], BF16, tag="expnegm")
            ytn = bigpool.tile([128, 2, T], BF16, tag="ytn")
            nc.gpsimd.memset(ytn[64:128, 1, :], 0.0)
            nc.gpsimd.memset(ytn[64:65, 1, :], 1.0)   # outproj bias row

            wpool = tc.alloc_tile_pool(name="work", bufs=2)
            spool = tc.alloc_tile_pool(name="small", bufs=5)
            ppool = tc.alloc_tile_pool(name="pstage", bufs=16)
            smpool = tc.alloc_tile_pool(name="sm2", bufs=2)
            xtpool = tc.alloc_tile_pool(name="xtp", bufs=1)
            xt = xtpool.tile([128, CT, T], BF16, tag="xt")

            nc.sync.dma_start(wqk[:], wqk_d[:])
            nc.sync.dma_start(bqk[:], bqk_d[:])
            for nqc in range(NQ):
                nc.sync.dma_start(xt[:, :, nqc * 512:(nqc + 1) * 512],
                                  xt_d[:, :, nqc * 512:(nqc + 1) * 512])
            for sb, dr in ((tris, tris_d), (trie, trie_d), (wv, wv_d),
                           (wp, wp_d)):
                nc.sync.dma_start(sb[:], dr[:])

            qk_dst = [qTp, kTp, qX, kX]

            def qk_group(nqc, mt):
                n0 = nqc * 512
                ps = psA.tile([128, 512], F32, tag="mm")
                for ct in range(CT):
                    nc.tensor.matmul(
                        ps[:],
                        wqk[:, ct, mt * 128:(mt + 1) * 128],
                        xt[:, ct, n0:n0 + 512],
                        start=(ct == 0), stop=(ct == CT - 1),
                        skip_group_check=True,
                    )
                nc.scalar.activation(qk_dst[mt][:, n0:n0 + 512], ps[:],
                                     AF.Identity, bias=bqk[:, mt:mt + 1])

            # qk projections; ALL of qX/kX (mt 2,3) first so the FF pipeline
            # for tiles 0-3 (own PSUM bank) overlaps the qTp/kTp projections
            for nqc in range(NQ):
                for mt in (2, 3):
                    qk_group(nqc, mt)
            # protect_bos_token: zero key 0's k_0 vector so S[key0, :] == 0
            nc.vector.memset(kX[64:128, 0:1], 0.0)

            def v_proj(tt):
                # single slot: PSUM slots are bank-granular; v chains are
                # sparse enough that serializing on one slot is cheap
                ps = psV.tile([128, 256], F32, tag="vps", bufs=1)
                for ct in range(CT):
                    nc.tensor.matmul(
                        ps[:, :HPG * D],
                        xt[:, ct, tt * 128:(tt + 1) * 128],
                        wv[:, ct, :],
                        start=(ct == 0), stop=(ct == CT - 1),
                        skip_group_check=True,
                    )
                nc.scalar.copy(vaug[:, tt, :, :D],
                               ps[:, :HPG * D].rearrange(
                                   "p (h x) -> p h x", h=HPG))

            def ff_tile(kt):
                """Generator of FF steps for one key tile: matmul chunks
                (PE rows 64-127, pairable with h2 att), relu (DVE diag
                chunk via tris512 mask, ACT for the tail), exclusive scan
                (DVE), exp (ACT), inclusive-triangle mask (DVE)."""
                base = kt * 128
                span = T - base
                s_sb = wpool.tile([128, T], BF16, tag="s_sb")
                for c0 in range(0, span, 512):
                    cw = min(512, span - c0)

                    def mm(c0=c0, cw=cw):
                        # dedicated PSUM bank: decouples the FF pipeline
                        # from the pair/h2 unit slot rotation in psA
                        ps0 = psV.tile([128, 512], F32, tag="ffc", bufs=1)
                        nc.tensor.matmul(
                            ps0[:, :cw],
                            kX[64:128, base:base + 128],
                            qX[64:128, base + c0:base + c0 + cw],
                            start=True, stop=True,
                        )
                        if c0 == 0:
                            # diagonal chunk: S = relu(att0) * strict-tri mask
                            # (tris is 1.0 beyond the first 128 cols)
                            nc.vector.scalar_tensor_tensor(
                                s_sb[:, 0:cw], ps0[:, 0:cw], 0.0, tris[:, :cw],
                                op0=OP.max, op1=OP.mult,
                            )
                        else:
                            nc.scalar.activation(
                                s_sb[:, c0:c0 + cw], ps0[:, :cw], AF.Relu)
                    yield mm

                def tail():
                    fft = wpool.tile([128, T], BF16, tag="fft")
                    nc.vector.memset(fft[:, 0:1], 0.0)
                    # exclusive prefix sum over queries; op1=max with
                    # data1=data0 is identity (state >= each nonneg element)
                    nc.vector.tensor_tensor_scan(
                        fft[:, 1:span], s_sb[:, 0:span - 1], s_sb[:, 0:span - 1],
                        initial=0.0, op0=OP.add, op1=OP.max,
                    )
                    nc.scalar.activation(
                        expnegm[:, kt, base:T], fft[:, :span], AF.Exp,
                        scale=-1.0)
                    # causal mask on the diagonal block (inclusive triangle)
                    nc.vector.tensor_mul(
                        expnegm[:, kt, base:base + 128],
                        expnegm[:, kt, base:base + 128], trie)
                yield tail

            def run_gen(g):
                for step in g:
                    step()

            def off_of(qc, kt):
                return 128 * max(0, kt - 4 * qc)

            def att_step_pair(qc, kt, ps_list):
                n0 = qc * 512
                off = off_of(qc, kt)
                w = 512 - off
                attp = psA.tile([128, 2, 512], F32, tag="mm")
                nc.tensor.matmul(
                    attp[:, 0, off:512],
                    kTp[0:64, kt * 128:(kt + 1) * 128],
                    qTp[0:64, n0 + off:n0 + 512],
                    start=True, stop=True, skip_group_check=True,
                    tile_position=(0, 0),
                )
                nc.tensor.matmul(
                    attp[:, 1, off:512],
                    kTp[64:128, kt * 128:(kt + 1) * 128],
                    qTp[64:128, n0 + off:n0 + 512],
                    start=True, stop=True, skip_group_check=True,
                    tile_position=(64, 0),
                )
                ea = spool.tile([128, 2, 512], BF16, tag="ea")
                nc.scalar.activation(ea[:, :, off:512], attp[:, :, off:512],
                                     AF.Exp)
                p = ppool.tile([128, 2, 512], BF16, tag="p")
                em = expnegm[:, kt:kt + 1, n0 + off:n0 + 512].to_broadcast(
                    [128, 2, w])
                nc.vector.tensor_mul(p[:, :, off:512], ea[:, :, off:512], em)
                ps_list.append((p, off))

            def att_step_h2(qc, kt0, ps_list, ffpull):
                n0 = qc * 512
                offs = (off_of(qc, kt0), off_of(qc, kt0 + 1))
                attp = psA.tile([128, 2, 512], F32, tag="mm")
                for i in range(2):
                    kt = kt0 + i
                    nc.tensor.matmul(
                        attp[:, i, offs[i]:512],
                        kX[0:64, kt * 128:(kt + 1) * 128],
                        qX[0:64, n0 + offs[i]:n0 + 512],
                        start=True, stop=True, skip_group_check=True,
                    )
                    ffpull()  # adjacent FF matmul -> row-band overlap
                ea = spool.tile([128, 2, 512], BF16, tag="ea")
                p = ppool.tile([128, 2, 512], BF16, tag="p")
                if offs[0] == 0 and offs[1] == 0:
                    nc.scalar.activation(ea[:], attp[:], AF.Exp)
                    nc.vector.tensor_mul(
                        p[:], ea[:], expnegm[:, kt0:kt0 + 2, n0:n0 + 512])
                else:
                    for i in range(2):
                        o = offs[i]
                        nc.scalar.activation(ea[:, i, o:512],
                                             attp[:, i, o:512], AF.Exp)
                        nc.vector.tensor_mul(
                            p[:, i, o:512], ea[:, i, o:512],
                            expnegm[:, kt0 + i, n0 + o:n0 + 512])
                ps_list.append((p, offs))

            def normalize(qc, h, yacc):
                n0 = qc * 512
                ssum = smpool.tile([1, 512], F32, tag="ssum")
                nc.scalar.copy(ssum[:], yacc[64:65, :])
                recip = smpool.tile([1, 512], F32, tag="recip")
                nc.vector.reciprocal_approx_fast(recip[:], ssum[:])
                rb = smpool.tile([64, 512], F32, tag="rb")
                nc.gpsimd.partition_broadcast(rb[:], recip[:])
                prow = (h * D) % 128
                pct = (h * D) // 128
                nc.vector.tensor_mul(
                    ytn[prow:prow + D, pct, n0:n0 + 512],
                    yacc[0:D, :],
                    rb[:],
                )

            def av_unit_pair(qc, ps_list):
                nkt = 4 * qc + 4
                yacc0 = psY.tile([128, 512], F32, tag="yacc")
                yacc1 = psY.tile([128, 512], F32, tag="yacc")
                for kt in range(nkt):
                    def step(kt=kt):
                        p, off = ps_list[kt]
                        for h, yacc in ((0, yacc0), (1, yacc1)):
                            nc.tensor.matmul(
                                yacc[0:65, off:512], vaug[:, kt, h, :],
                                p[:, h, off:512],
                                start=(kt == 0), stop=(kt == nkt - 1),
                                skip_group_check=True,
                            )
                    yield step
                yield lambda: normalize(qc, 0, yacc0)
                yield lambda: normalize(qc, 1, yacc1)

            def av_unit_h2(qc, ps_list):
                nkt = 4 * qc + 4
                yacc = psY.tile([128, 512], F32, tag="yacc")
                for kt in range(nkt):
                    def step(kt=kt):
                        p, offs = ps_list[kt // 2]
                        off = offs[kt % 2]
                        nc.tensor.matmul(
                            yacc[0:65, off:512], vaug[:, kt, 2, :],
                            p[:, kt % 2, off:512],
                            start=(kt == 0), stop=(kt == nkt - 1),
                            skip_group_check=True,
                        )
                    yield step
                yield lambda: normalize(qc, 2, yacc)

            def outproj(qc):
                n0 = qc * 512
                for mc in range(6):
                    def step(mc=mc):
                        ops_ = psY.tile([128, 512], F32, tag="yacc")
                        for c2 in range(2):
                            nc.tensor.matmul(
                                ops_[:],
                                wp[:, c2, mc * 128:(mc + 1) * 128],
                                ytn[:, c2, n0:n0 + 512],
                                start=(c2 == 0), stop=(c2 == 1),
                                skip_group_check=True,
                            )
                        osb = smpool.tile([128, 512], F32, tag="osb")
                        nc.vector.tensor_copy(osb[:], ops_[:])
                        nc.sync.dma_start(
                            out_d[mc * 128:(mc + 1) * 128, n0:n0 + 512],
                            osb[:])
                    yield step

            # ---- prologue: qTp/kTp proj with ff 0..3 overlapped ----
            from collections import deque
            ffpre = deque()
            for kt in range(4):
                ffpre.extend(ff_tile(kt))
            for nqc in range(NQ):
                for mt in (0, 1):
                    qk_group(nqc, mt)
                    for _ in range(3):
                        if ffpre:
                            ffpre.popleft()()
            while ffpre:
                ffpre.popleft()()
            for kt in range(4):
                v_proj(kt)

            # ---- software-pipelined emission ----
            ffgens = {kt: ff_tile(kt) for kt in range(4, KT)}
            pending = deque()

            def drain(k):
                for _ in range(k):
                    if not pending:
                        return
                    pending.popleft()()

            ffq = deque(sorted(ffgens))        # 4..15
            ffcur = []                          # steps of the active ff gen

            def ffpull(n=1):
                for _ in range(n):
                    if not ffcur and ffq:
                        ffcur.extend(ffgens.pop(ffq.popleft()))
                    if ffcur:
                        ffcur.pop(0)()

            def ff_flush_through(kt_hi):
                while ffcur or (ffq and ffq[0] <= kt_hi):
                    ffpull()

            vq = deque(range(4, KT))
            for qc in range(NQ):
                nkt = 4 * qc + 4
                # pair unit
                ps_list = []
                for kt in range(nkt):
                    att_step_pair(qc, kt, ps_list)
                    drain(3)
                pending.extend(av_unit_pair(qc, ps_list))
                for _ in range(2):
                    if vq:
                        v_proj(vq.popleft())
                # h2 unit (+ paired ff matmuls)
                ps_list = []
                for kt0 in range(0, nkt, 2):
                    att_step_h2(qc, kt0, ps_list, ffpull)
                    drain(2)
                    ffpull(2)
                ff_flush_through(4 * qc + 7)
                pending.extend(av_unit_h2(qc, ps_list))
                for _ in range(2):
                    if vq:
                        v_proj(vq.popleft())
                pending.extend(outproj(qc))
            while pending:
                pending.popleft()()

            xtpool.release()
            smpool.release()
            ppool.release()
            spool.release()
            wpool.release()

    nc.compile()
    return nc


def _pad_ct(a, ct):
    """[rows<=ct*128, n] -> [128, ct, n]."""
    n = a.shape[1]
    out = np.zeros((ct * 128, n), a.dtype)
    out[:a.shape[0]] = a
    return np.ascontiguousarray(out.reshape(ct, 128, n).transpose(1, 0, 2))


def _prep_inputs(x, w_attn, b_attn, w_proj, b_proj):
    """Build the 8 per-core input maps."""
    scale = np.float32(1.0 / math.sqrt(D))
    HD = H * D
    bf = ml_dtypes.bfloat16

    w_q = (w_attn[:, :HD] * scale).astype(np.float32)
    b_q = (b_attn[:HD] * scale).astype(np.float32)
    w_k, b_k = w_attn[:, HD:2 * HD], b_attn[HD:2 * HD]
    w_v, b_v = w_attn[:, 2 * HD:], b_attn[2 * HD:]

    r = np.arange(128)
    tris = np.ones((128, 512), np.float32)
    tris[:, :128] = (r[None, :] > r[:, None])                # query > key
    trie = (r[None, :] >= r[:, None]).astype(np.float32)     # query >= key

    maps = []
    for core in range(N_CORES):
        b, g = divmod(core, G)
        h0 = g * HPG * D
        # wqk col layout: [q_h0|q_h1][k_h0|k_h1][q_h2|q_0][k_h2|k_0]
        wqk = np.hstack([
            w_q[:, h0:h0 + 2 * D], w_k[:, h0:h0 + 2 * D],
            w_q[:, h0 + 2 * D:h0 + 3 * D], w_q[:, :D],
            w_k[:, h0 + 2 * D:h0 + 3 * D], w_k[:, :D],
        ])  # [768, 512]
        bqk = np.stack([
            np.concatenate([b_q[h0:h0 + D], b_q[h0 + D:h0 + 2 * D]]),
            np.concatenate([b_k[h0:h0 + D], b_k[h0 + D:h0 + 2 * D]]),
            np.concatenate([b_q[h0 + 2 * D:h0 + 3 * D], b_q[:D]]),
            np.concatenate([b_k[h0 + 2 * D:h0 + 3 * D], b_k[:D]]),
        ], axis=1).astype(np.float32)  # [128, 4]
        wp_rows = np.zeros((256, C), np.float32)
        wp_rows[:HPG * D] = w_proj[h0:h0 + HPG * D]
        # bias row at contraction position 192 (row 64 of ct-1), matching the
        # ones row in ytn: per-core v-bias fold + b_proj on group 0 only
        bias_vec = b_v[h0:h0 + HPG * D] @ w_proj[h0:h0 + HPG * D]
        if g == 0:
            bias_vec = bias_vec + b_proj
        wp_rows[HPG * D] = bias_vec
        maps.append({
            "xt": _pad_ct(x[b].T, CT).astype(bf),
            "wqk": _pad_ct(wqk, CT).astype(bf),
            "bqk": bqk,
            "wv": _pad_ct(w_v[:, h0:h0 + HPG * D], CT).astype(bf),
            "wp": np.ascontiguousarray(
                wp_rows.reshape(2, 128, C).transpose(1, 0, 2)).astype(bf),
            "tris": tris.astype(bf),
            "trie": trie.astype(bf),
        })
    return maps


LAST_RESULTS = None


def kernel(x, w_attn, b_attn, w_proj, b_proj):
    global LAST_RESULTS
    x = np.asarray(x, np.float32)
    w_attn = np.asarray(w_attn, np.float32)
    b_attn = np.asarray(b_attn, np.float32)
    w_proj = np.asarray(w_proj, np.float32)
    b_proj = np.asarray(b_proj, np.float32)

    if "nc" not in _CACHED:
        _CACHED["nc"] = build_program()
    nc = _CACHED["nc"]

    in_maps = _prep_inputs(x, w_attn, b_attn, w_proj, b_proj)
    res = run_bass_kernel_spmd(
        nc, in_maps, core_ids=list(range(N_CORES)),
        trace=bool(os.environ.get("KERNEL_TRACE")),
    )
    LAST_RESULTS = res

    out = np.zeros((B, T, C), np.float32)
    for core in range(N_CORES):
        b = core // G
        out[b] += res.results[core]["out"].T
    return out


if __name__ == "__main__":
    rng = np.random.default_rng(0)
    x = rng.standard_normal((B, T, C), np.float32)
    s = 1.0 / math.sqrt(C)
    w_attn = rng.uniform(-s, s, (C, 3 * H * D)).astype(np.float32)
    b_attn = rng.uniform(-s, s, (3 * H * D,)).astype(np.float32)
    sp = 1.0 / math.sqrt(H * D)
    w_proj = rng.uniform(-sp, sp, (H * D, C)).astype(np.float32)
    b_proj = rng.uniform(-sp, sp, (C,)).astype(np.float32)
    y = kernel(x=x, w_attn=w_attn, b_attn=b_attn, w_proj=w_proj, b_proj=b_proj)
    print("out", y.shape, float(np.abs(y).mean()))

